# revision 1
# baseline (speedup 1.0000x reference)
"""Masked-softmax attention pooling on 8 TRN2 NeuronCores.

Reference computation (per batch b):
    q = hidden @ W.T                      # [H]
    alphas[s] = eo[b, s, :] . q           # [S]
    alphas = where(mask, -1e16, alphas)
    scores = softmax(alphas)              # over S
    out[b] = sum_s scores[s] * eo[b, s, :]

Sharding: data-parallel over batch (8 batches/core), W replicated.
encoder_output dominates traffic (64 MiB/core) and is streamed from HBM
exactly once in its natural [S, H] layout:
  - alphas via fused DVE multiply+reduce against a partition-broadcast q
  - weighted sum via TensorE with the score column as the stationary operand
"""

from contextlib import ExitStack

import numpy as np

import concourse.bass as bass
import concourse.tile as tile
from concourse import bacc, mybir
from concourse._compat import get_trn_type
from concourse.bass_utils import run_bass_kernel_spmd
from concourse.masks import make_identity

B, S, H = 64, 2048, 1024
N_CORES = 8
BL = B // N_CORES      # 8 batches per core
SC = S // 128          # 16 s-chunks per batch
HC = H // 128          # 8 h-chunks
F32 = mybir.dt.float32

NEG_BIG = -1.0e16
CLAMP = -100.0         # exp(CLAMP) == 0 in f32 for softmax purposes
EO_BUFS = 24           # >= SC (all chunks of one batch live) + prefetch slots


def _build(BL=BL, S=S, H=H, EO_BUFS=EO_BUFS, stage=4):
    SC = S // 128
    HC = H // 128
    nc = bacc.Bacc(get_trn_type() or "TRN2", target_bir_lowering=False)

    hid_d = nc.dram_tensor("hidden", [BL, H], F32, kind="ExternalInput")
    eo_d = nc.dram_tensor("encoder_output", [BL, S, H], F32, kind="ExternalInput")
    mk_d = nc.dram_tensor("encoder_mask", [BL, S], F32, kind="ExternalInput")
    w_d = nc.dram_tensor("W", [H, H], F32, kind="ExternalInput")
    out_d = nc.dram_tensor("out", [BL, H], F32, kind="ExternalOutput")

    with tile.TileContext(nc) as tc, ExitStack() as ctx:
        sing = ctx.enter_context(tc.tile_pool(name="sing", bufs=1))
        wpool = ctx.enter_context(tc.tile_pool(name="wpool", bufs=2))
        eop = ctx.enter_context(tc.tile_pool(name="eop", bufs=EO_BUFS))
        qbp = ctx.enter_context(tc.tile_pool(name="qbp", bufs=2))
        prodp = ctx.enter_context(tc.tile_pool(name="prodp", bufs=2))
        smallp = ctx.enter_context(tc.tile_pool(name="smallp", bufs=2))
        outp = ctx.enter_context(tc.tile_pool(name="outp", bufs=2))
        ps1 = ctx.enter_context(tc.tile_pool(name="ps1", bufs=4, space="PSUM"))
        ps2 = ctx.enter_context(tc.tile_pool(name="ps2", bufs=2, space="PSUM"))
        dramp = ctx.enter_context(tc.tile_pool(name="dramp", bufs=1, space="DRAM"))

        if stage < 1:
            # stage 0: just stream eo in and write a constant out
            for b in range(BL):
                for c in range(SC):
                    x = eop.tile([128, H], F32, tag="x")
                    nc.sync.dma_start(x[:], eo_d[b, bass.ts(c, 128), :])
                    nc.scalar.mul(x[:], x[:], 2.0)
                c_sb = outp.tile([1, H], F32, tag="c_sb")
                nc.vector.memset(c_sb[:], 7.0)
                nc.sync.dma_start(out_d[b : b + 1, :], c_sb[:])
            return nc

        # ---- constants
        ident = sing.tile([128, 128], F32)
        make_identity(nc, ident[:])
        zero1 = sing.tile([128, 1], F32)
        nc.vector.memset(zero1[:], 0.0)
        one1 = sing.tile([128, 1], F32)
        nc.vector.memset(one1[:], 1.0)

        # ---- prologue: q = hidden @ W.T   (contract over h; need h on partitions)
        hid = sing.tile([BL, H], F32)
        nc.sync.dma_start(hid[:], hid_d[:])
        hT = sing.tile([128, HC, BL], F32)  # hidden^T, per h-chunk [128h, BL]
        for h in range(HC):
            p = ps1.tile([128, BL], F32, tag="ps1")
            nc.tensor.transpose(p[:], hid[:, bass.ts(h, 128)], ident[0:BL, 0:BL])
            nc.scalar.copy(hT[:, h, :], p[:])

        wt = sing.tile([128, HC, H], F32)   # W^T, per h-chunk [128h, 1024o]
        for o in range(HC):
            wrow = wpool.tile([128, H], F32, tag="wrow")
            nc.sync.dma_start(wrow[:], w_d[bass.ts(o, 128), :])
            for h in range(HC):
                p = ps1.tile([128, 128], F32, tag="ps1")
                nc.tensor.transpose(p[:], wrow[:, bass.ts(h, 128)], ident[:])
                nc.scalar.copy(wt[:, h, bass.ts(o, 128)], p[:])

        q_ps = ps2.tile([BL, H], F32, tag="ps2")
        for nh in range(H // 512):
            for h in range(HC):
                nc.tensor.matmul(
                    q_ps[:, bass.ts(nh, 512)],
                    hT[:, h, :],
                    wt[:, h, bass.ts(nh, 512)],
                    start=(h == 0),
                    stop=(h == HC - 1),
                )
        q_sb = sing.tile([BL, H], F32)
        nc.scalar.copy(q_sb[:], q_ps[:])
        # bounce q through DRAM so it can be partition-broadcast per batch
        q_dram = dramp.tile([BL, H], F32)
        nc.sync.dma_start(q_dram[:], q_sb[:])

        # ---- mask, all batches at once: [SC, BL, 128] (partition = chunk)
        # onem = 1 - mask (1 at positions that participate in the softmax)
        mk_all = sing.tile([SC, BL, 128], F32)
        nc.gpsimd.dma_start(mk_all[:], mk_d.rearrange("b (c p) -> c b p", p=128))
        onem_all = sing.tile([SC, BL, 128], F32)
        nc.scalar.activation(
            out=onem_all[:], in_=mk_all[:],
            func=mybir.ActivationFunctionType.Identity,
            bias=one1[0:SC, :], scale=-1.0,
        )

        # ---- main loop over local batches
        for b in range(BL):
            qb = qbp.tile([128, H], F32, tag="qb")
            nc.gpsimd.dma_start(qb[:], q_dram[b : b + 1, :].to_broadcast([128, H]))

            xs = []
            alphas = smallp.tile([128, SC], F32, tag="alphas")
            for c in range(SC):
                x = eop.tile([128, H], F32, tag="x")
                nc.sync.dma_start(x[:], eo_d[b, bass.ts(c, 128), :])
                xs.append(x)
                if stage < 2:
                    continue
                prod = prodp.tile([128, H], F32, tag="prod")
                nc.vector.tensor_mul(prod[:], x[:], qb[:])
                # reduce on ScalarE (Copy + accumulate) to keep DVE free
                trash = prodp.tile([128, H], F32, tag="trash")
                nc.scalar.activation(
                    out=trash[:], in_=prod[:],
                    func=mybir.ActivationFunctionType.Copy,
                    bias=0.0, scale=1.0,
                    accum_out=alphas[:, c : c + 1],
                )

            if stage < 3:
                # debug probe: output = broadcast q row (validates everything
                # up to and including alphas structurally)
                c_sb = outp.tile([1, H], F32, tag="c_sb")
                nc.scalar.copy(c_sb[:], qb[0:1, :])
                nc.sync.dma_start(out_d[b : b + 1, :], c_sb[:])
                continue

            # two-level softmax without cross-partition broadcasts or -inf fills:
            #   m1[p] = max_c alphas[p,c]   (raw; >= unmasked max, exps stay <= 1)
            #   u[p,c] = exp(alphas - m1[p]) * (1 - mask)   (exact zero at masked)
            #   g[p]   = exp(m1[p] - mx)                    (fix-up factor)
            m1 = smallp.tile([128, 1], F32, tag="m1")
            nc.vector.tensor_reduce(
                out=m1[:], in_=alphas[:],
                axis=mybir.AxisListType.X, op=mybir.AluOpType.max,
            )
            negm1 = smallp.tile([128, 1], F32, tag="negm1")
            nc.scalar.mul(negm1[:], m1[:], -1.0)
            e = smallp.tile([128, SC], F32, tag="e")
            nc.scalar.activation(
                out=e[:], in_=alphas[:],
                func=mybir.ActivationFunctionType.Exp,
                bias=negm1[:], scale=1.0,
            )
            onemp = ps1.tile([128, SC], F32, tag="ps1")
            nc.tensor.transpose(onemp[:], onem_all[:, b, :], ident[0:SC, 0:SC])
            u = smallp.tile([128, SC], F32, tag="u")
            nc.vector.tensor_mul(u[:], e[:], onemp[:])
            s1 = smallp.tile([128, 1], F32, tag="s1")
            nc.vector.tensor_reduce(
                out=s1[:], in_=u[:],
                axis=mybir.AxisListType.X, op=mybir.AluOpType.add,
            )

            # transpose m1 and s1 to one partition each
            m1p = ps1.tile([1, 128], F32, tag="ps1")
            nc.tensor.transpose(m1p[:], m1[:], ident[:])
            s1p = ps1.tile([1, 128], F32, tag="ps1")
            nc.tensor.transpose(s1p[:], s1[:], ident[:])
            mx = smallp.tile([1, 1], F32, tag="mx")
            nc.vector.tensor_reduce(
                out=mx[:], in_=m1p[:],
                axis=mybir.AxisListType.X, op=mybir.AluOpType.max,
            )
            negmx = smallp.tile([1, 1], F32, tag="negmx")
            nc.scalar.mul(negmx[:], mx[:], -1.0)
            d = smallp.tile([1, 128], F32, tag="d")
            nc.vector.tensor_scalar_add(d[:], m1p[:], negmx[0:1, 0:1])
            dc = smallp.tile([1, 128], F32, tag="dc")
            nc.vector.tensor_scalar_max(dc[:], d[:], CLAMP)
            g = smallp.tile([1, 128], F32, tag="g")
            nc.scalar.activation(
                out=g[:], in_=dc[:],
                func=mybir.ActivationFunctionType.Exp,
                bias=zero1[0:1, :], scale=1.0,
            )
            w = smallp.tile([1, 128], F32, tag="w")
            nc.vector.tensor_mul(w[:], s1p[:], g[:])
            den = smallp.tile([1, 1], F32, tag="den")
            nc.vector.tensor_reduce(
                out=den[:], in_=w[:],
                axis=mybir.AxisListType.X, op=mybir.AluOpType.add,
            )
            r = smallp.tile([1, 1], F32, tag="r")
            nc.vector.reciprocal(r[:], den[:])

            # scale u rows by g (transpose g back to [128, 1] first)
            gp = ps1.tile([128, 1], F32, tag="ps1")
            nc.tensor.transpose(gp[:], g[:], ident[0:1, 0:1])
            us = smallp.tile([128, SC], F32, tag="us")
            nc.vector.tensor_scalar_mul(us[:], u[:], gp[:])

            # c = (sum_s us[s] * eo[s, :]) / den   via TensorE, us column stationary
            c_ps = ps2.tile([1, H], F32, tag="ps2")
            for c in range(SC):
                for nh in range(H // 512):
                    nc.tensor.matmul(
                        c_ps[0:1, bass.ts(nh, 512)],
                        us[:, c : c + 1],
                        xs[c][:, bass.ts(nh, 512)],
                        start=(c == 0),
                        stop=(c == SC - 1),
                    )
            c_sb = outp.tile([1, H], F32, tag="c_sb")
            nc.scalar.activation(
                out=c_sb[:], in_=c_ps[:],
                func=mybir.ActivationFunctionType.Copy,
                bias=0.0, scale=r[0:1, 0:1],
            )
            nc.sync.dma_start(out_d[b : b + 1, :], c_sb[:])

    nc.compile()
    return nc


_CACHE = {}


def _get_nc():
    if "nc" not in _CACHE:
        _CACHE["nc"] = _build()
    return _CACHE["nc"]


def _make_in_maps(hidden, encoder_output, encoder_mask, W):
    hidden = np.ascontiguousarray(hidden, dtype=np.float32)
    eo = np.ascontiguousarray(encoder_output, dtype=np.float32)
    mk = np.ascontiguousarray(
        encoder_mask.reshape(B, S).astype(np.float32)
    )
    W = np.ascontiguousarray(W, dtype=np.float32)
    in_maps = []
    for i in range(N_CORES):
        sl = slice(i * BL, (i + 1) * BL)
        in_maps.append(
            {
                "hidden": hidden[sl],
                "encoder_output": eo[sl],
                "encoder_mask": mk[sl],
                "W": W,
            }
        )
    return in_maps


def run(hidden, encoder_output, encoder_mask, W, trace=False):
    nc = _get_nc()
    in_maps = _make_in_maps(hidden, encoder_output, encoder_mask, W)
    res = run_bass_kernel_spmd(nc, in_maps, list(range(N_CORES)), trace=trace)
    out = np.concatenate([res.results[i]["out"] for i in range(N_CORES)], axis=0)
    return out, res


def kernel(hidden, encoder_output, encoder_mask, W):
    out, _ = run(hidden, encoder_output, encoder_mask, W, trace=False)
    return out



# revision 8
# speedup vs baseline: 1.3372x; 1.3372x over previous
"""Masked-softmax attention pooling on 8 TRN2 NeuronCores.

Reference computation (per batch b):
    q = hidden @ W.T                      # [H]
    alphas[s] = eo[b, s, :] . q           # [S]
    alphas = where(mask, -1e16, alphas)
    scores = softmax(alphas)              # over S
    out[b] = sum_s scores[s] * eo[b, s, :]

Sharding: data-parallel over batch (8 batches/core), W replicated.
encoder_output dominates traffic (64 MiB/core) and is streamed from HBM
exactly once in its natural [S, H] layout:
  - alphas via ONE fused DVE op per 128-row chunk (tensor_tensor_reduce:
    accum_out = sum_h x*q)
  - the fused op also emits prod = x*q in bf16; the weighted sum runs
    on TensorE over prod at full bf16 column rate (4x faster than
    fp32), and the output row is divided by q at the end:
    sum_s w_s*(x_s*q)/q == sum_s w_s*x_s (bf16 error enters linearly)
"""

from contextlib import ExitStack

import numpy as np

import concourse.bass as bass
import concourse.tile as tile
from concourse import bacc, mybir
from concourse._compat import get_trn_type
from concourse.bass_utils import run_bass_kernel_spmd
from concourse.masks import make_identity

B, S, H = 64, 2048, 1024
N_CORES = 8
BL = B // N_CORES      # 8 batches per core
SC = S // 128          # 16 s-chunks per batch
HC = H // 128          # 8 h-chunks
F32 = mybir.dt.float32
F32R = mybir.dt.float32r
BF16 = mybir.dt.bfloat16

EO_BUFS = 10           # x is freed right after the fused DVE op
PROD_BUFS = 32         # bf16 product tiles; live until the batch's matmuls


def _build():
    nc = bacc.Bacc(get_trn_type() or "TRN2", target_bir_lowering=False)

    hid_d = nc.dram_tensor("hidden", [BL, H], F32, kind="ExternalInput")
    eo_d = nc.dram_tensor("encoder_output", [BL, S, H], F32, kind="ExternalInput")
    mk_d = nc.dram_tensor("encoder_mask", [BL, S], F32, kind="ExternalInput")
    w_d = nc.dram_tensor("W", [H, H], F32, kind="ExternalInput")
    out_d = nc.dram_tensor("out", [BL, H], F32, kind="ExternalOutput")

    MULT = mybir.AluOpType.mult
    ADD = mybir.AluOpType.add
    MAX = mybir.AluOpType.max
    AX = mybir.AxisListType.X
    EXP = mybir.ActivationFunctionType.Exp
    IDN = mybir.ActivationFunctionType.Identity
    CPY = mybir.ActivationFunctionType.Copy

    with tile.TileContext(nc) as tc, ExitStack() as ctx:
        sing = ctx.enter_context(tc.tile_pool(name="sing", bufs=1))
        wpool = ctx.enter_context(tc.tile_pool(name="wpool", bufs=2))
        eop = ctx.enter_context(tc.tile_pool(name="eop", bufs=EO_BUFS))
        prodp = ctx.enter_context(tc.tile_pool(name="prodp", bufs=PROD_BUFS))
        tmpp = ctx.enter_context(tc.tile_pool(name="tmpp", bufs=2))
        invqp = ctx.enter_context(tc.tile_pool(name="invqp", bufs=2))
        qbp = ctx.enter_context(tc.tile_pool(name="qbp", bufs=3))
        alphp = ctx.enter_context(tc.tile_pool(name="alphp", bufs=3))
        smallp = ctx.enter_context(tc.tile_pool(name="smallp", bufs=3))
        outp = ctx.enter_context(tc.tile_pool(name="outp", bufs=1))
        psS = ctx.enter_context(tc.tile_pool(name="psS", bufs=4, space="PSUM"))
        psB = ctx.enter_context(tc.tile_pool(name="psB", bufs=2, space="PSUM"))
        dramp = ctx.enter_context(tc.tile_pool(name="dramp", bufs=1, space="DRAM"))

        # ---- constants
        ident = sing.tile([128, 128], F32)
        make_identity(nc, ident[:])
        one1 = sing.tile([128, 1], F32)
        nc.vector.memset(one1[:], 1.0)

        # ---- W^T: wt[:, h, o] = W[o, h]  (h on partitions)
        # 4 TensorE transposes packed per PSUM bank -> 1 scalar copy each
        wt = sing.tile([128, HC, H], F32)
        for o in range(HC):
            wrow = wpool.tile([128, H], F32, tag="wrow")
            nc.sync.dma_start(wrow[:], w_d[bass.ts(o, 128), :])
            for hb in range(2):
                p4 = psS.tile([128, 4, 128], F32, tag="psS")
                for j in range(4):
                    h = hb * 4 + j
                    nc.tensor.transpose(p4[:, j, :], wrow[:, bass.ts(h, 128)], ident[:])
                nc.scalar.copy(wt[:, hb * 4 : hb * 4 + 4, bass.ts(o, 128)], p4[:])

        # ---- hidden^T, per h-chunk [128h, BL]
        hid = sing.tile([BL, H], F32)
        nc.sync.dma_start(hid[:], hid_d[:])
        pH = psS.tile([128, HC, BL], F32, tag="psS")
        for h in range(HC):
            nc.tensor.transpose(pH[:, h, :], hid[:, bass.ts(h, 128)], ident[0:BL, 0:BL])
        hT = sing.tile([128, HC, BL], F32)
        nc.scalar.copy(hT[:], pH[:])

        # ---- q = hidden @ W.T   (f32 for alphas precision)
        q_ps = psB.tile([BL, H], F32, tag="big")
        for nh in range(H // 512):
            for h in range(HC):
                nc.tensor.matmul(
                    q_ps[:, bass.ts(nh, 512)],
                    hT[:, h, :],
                    wt[:, h, bass.ts(nh, 512)],
                    start=(h == 0),
                    stop=(h == HC - 1),
                )
        q_sb = sing.tile([BL, H], F32)
        nc.scalar.copy(q_sb[:], q_ps[:])
        # bounce q through DRAM so it can be partition-broadcast per batch
        q_dram = dramp.tile([BL, H], F32)
        nc.sync.dma_start(q_dram[:], q_sb[:])
        invq_all = sing.tile([BL, H], F32)
        nc.vector.reciprocal(invq_all[:], q_sb[:])
        invq_dram = dramp.tile([BL, H], F32)
        nc.sync.dma_start(invq_dram[:], invq_all[:])

        # ---- mask, all batches at once: [SC, BL, 128] (partition = chunk)
        mk_all = sing.tile([SC, BL, 128], F32)
        nc.gpsimd.dma_start(mk_all[:], mk_d.rearrange("b (c p) -> c b p", p=128))
        onem_all = sing.tile([SC, BL, 128], F32)
        nc.scalar.activation(
            out=onem_all[:], in_=mk_all[:],
            func=IDN, bias=one1[0:SC, :], scale=-1.0,
        )

        # ---- main loop over local batches
        for b in range(BL):
            qb = qbp.tile([128, H], F32, tag="qb")
            nc.gpsimd.dma_start(qb[:], q_dram[b : b + 1, :].to_broadcast([128, H]))

            alphas = alphp.tile([128, SC], F32, tag="alphas")
            prods = []
            for c in range(SC):
                x = eop.tile([128, H], F32, tag="x")
                nc.sync.dma_start(x[:], eo_d[b, bass.ts(c, 128), :])
                ptmp = tmpp.tile([128, H], F32, tag="ptmp")
                nc.vector.tensor_mul(ptmp[:], x[:], qb[:])
                prod = prodp.tile([128, H], BF16, tag="prod")
                # ScalarE: cast product to bf16 AND reduce alphas in one pass
                nc.scalar.activation(
                    out=prod[:], in_=ptmp[:], func=CPY,
                    bias=0.0, scale=1.0,
                    accum_out=alphas[:, c : c + 1],
                )
                prods.append(prod)

            # two-level softmax: raw per-row max m1 keeps exps <= 1; the
            # transposed side computes the global fix-up g = exp(m1 - mx).
            m1 = smallp.tile([128, 1], F32, tag="m1")
            nc.vector.tensor_reduce(out=m1[:], in_=alphas[:], axis=AX, op=MAX)
            negm1 = smallp.tile([128, 1], F32, tag="negm1")
            nc.scalar.mul(negm1[:], m1[:], -1.0)
            e = smallp.tile([128, SC], F32, tag="e")
            nc.scalar.activation(out=e[:], in_=alphas[:], func=EXP,
                                 bias=negm1[:], scale=1.0)
            onemp = psS.tile([128, SC], F32, tag="psS")
            nc.tensor.transpose(onemp[:], onem_all[:, b, :], ident[0:SC, 0:SC])
            u = smallp.tile([128, SC], F32, tag="u")
            nc.vector.tensor_mul(u[:], e[:], onemp[:])
            s1 = smallp.tile([128, 1], F32, tag="s1")
            nc.vector.tensor_reduce(out=s1[:], in_=u[:], axis=AX, op=ADD)

            m1p = psS.tile([1, 128], F32, tag="psS")
            nc.tensor.transpose(m1p[:], m1[:], ident[:])
            s1p = psS.tile([1, 128], F32, tag="psS")
            nc.tensor.transpose(s1p[:], s1[:], ident[:])
            mx = smallp.tile([1, 1], F32, tag="mx")
            nc.vector.tensor_reduce(out=mx[:], in_=m1p[:], axis=AX, op=MAX)
            negmx = smallp.tile([1, 1], F32, tag="negmx")
            nc.scalar.mul(negmx[:], mx[:], -1.0)
            g = smallp.tile([1, 128], F32, tag="g")
            nc.scalar.activation(out=g[:], in_=m1p[:], func=EXP,
                                 bias=negmx[0:1, :], scale=1.0)
            w = smallp.tile([1, 128], F32, tag="w")
            nc.vector.tensor_mul(w[:], s1p[:], g[:])
            den = smallp.tile([1, 1], F32, tag="den")
            nc.vector.tensor_reduce(out=den[:], in_=w[:], axis=AX, op=ADD)
            r = smallp.tile([1, 1], F32, tag="r")
            nc.vector.reciprocal(r[:], den[:])
            gps = psS.tile([128, 1], F32, tag="psS")
            nc.tensor.transpose(gps[:], g[:], ident[0:1, 0:1])
            gp = smallp.tile([128, 1], F32, tag="gp")
            nc.scalar.copy(gp[:], gps[:])
            us = smallp.tile([128, SC], BF16, tag="us")
            nc.vector.tensor_scalar_mul(us[:], u[:], gp[:])

            # c*q = sum_c us[:, c]^T @ prod_c   (bf16, full PE rate)
            c_ps = psB.tile([1, H], F32, tag="big")
            for c in range(SC):
                for nh in range(H // 512):
                    nc.tensor.matmul(
                        c_ps[0:1, bass.ts(nh, 512)],
                        us[:, c : c + 1],
                        prods[c][:, bass.ts(nh, 512)],
                        start=(c == 0),
                        stop=(c == SC - 1),
                    )
            # c = (c*q) * r / q ; the bf16 error enters linearly only
            invqb = invqp.tile([1, H], F32, tag="invqb")
            nc.sync.dma_start(invqb[:], invq_dram[b : b + 1, :])
            c_sb = outp.tile([1, H], F32, tag="c_sb")
            nc.scalar.activation(out=c_sb[:], in_=c_ps[:], func=CPY,
                                 bias=0.0, scale=r[0:1, 0:1])
            final = outp.tile([1, H], F32, tag="final")
            nc.vector.tensor_mul(final[:], c_sb[:], invqb[:])
            nc.sync.dma_start(out_d[b : b + 1, :], final[:])

    nc.compile()
    return nc


_CACHE = {}


def _get_nc():
    if "nc" not in _CACHE:
        _CACHE["nc"] = _build()
    return _CACHE["nc"]


def _make_in_maps(hidden, encoder_output, encoder_mask, W):
    hidden = np.ascontiguousarray(hidden, dtype=np.float32)
    eo = np.ascontiguousarray(encoder_output, dtype=np.float32)
    mk = np.ascontiguousarray(
        encoder_mask.reshape(B, S).astype(np.float32)
    )
    W = np.ascontiguousarray(W, dtype=np.float32)
    in_maps = []
    for i in range(N_CORES):
        sl = slice(i * BL, (i + 1) * BL)
        in_maps.append(
            {
                "hidden": hidden[sl],
                "encoder_output": eo[sl],
                "encoder_mask": mk[sl],
                "W": W,
            }
        )
    return in_maps


def run(hidden, encoder_output, encoder_mask, W, trace=False):
    nc = _get_nc()
    in_maps = _make_in_maps(hidden, encoder_output, encoder_mask, W)
    res = run_bass_kernel_spmd(nc, in_maps, list(range(N_CORES)), trace=trace)
    out = np.concatenate([res.results[i]["out"] for i in range(N_CORES)], axis=0)
    return out, res


def kernel(hidden, encoder_output, encoder_mask, W):
    out, _ = run(hidden, encoder_output, encoder_mask, W, trace=False)
    return out


# revision 9
# speedup vs baseline: 1.4058x; 1.0513x over previous
"""Masked-softmax attention pooling on 8 TRN2 NeuronCores.

Reference computation (per batch b):
    q = hidden @ W.T                      # [H]
    alphas[s] = eo[b, s, :] . q           # [S]
    alphas = where(mask, -1e16, alphas)
    scores = softmax(alphas)              # over S
    out[b] = sum_s scores[s] * eo[b, s, :]

Sharding: data-parallel over batch (8 batches/core), W replicated.
encoder_output dominates traffic (64 MiB/core) and is streamed from HBM
exactly once in its natural [S, H] layout:
  - alphas via ONE fused DVE op per 128-row chunk (tensor_tensor_reduce:
    accum_out = sum_h x*q)
  - the fused op also emits prod = x*q in bf16; the weighted sum runs
    on TensorE over prod at full bf16 column rate (4x faster than
    fp32), and the output row is divided by q at the end:
    sum_s w_s*(x_s*q)/q == sum_s w_s*x_s (bf16 error enters linearly)
"""

from contextlib import ExitStack

import numpy as np

import concourse.bass as bass
import concourse.tile as tile
from concourse import bacc, mybir
from concourse._compat import get_trn_type
from concourse.bass_utils import run_bass_kernel_spmd
from concourse.masks import make_identity

B, S, H = 64, 2048, 1024
N_CORES = 8
BL = B // N_CORES      # 8 batches per core
SC = S // 128          # 16 s-chunks per batch
HC = H // 128          # 8 h-chunks
F32 = mybir.dt.float32
F32R = mybir.dt.float32r
BF16 = mybir.dt.bfloat16

EO_BUFS = 10           # x is freed right after the fused DVE op
PROD_BUFS = 32         # bf16 product tiles; live until the batch's matmuls


def _build():
    nc = bacc.Bacc(get_trn_type() or "TRN2", target_bir_lowering=False)

    hid_d = nc.dram_tensor("hidden", [BL, H], F32, kind="ExternalInput")
    eo_d = nc.dram_tensor("encoder_output", [BL, S, H], F32, kind="ExternalInput")
    mk_d = nc.dram_tensor("encoder_mask", [BL, S], F32, kind="ExternalInput")
    w_d = nc.dram_tensor("W", [H, H], F32, kind="ExternalInput")
    out_d = nc.dram_tensor("out", [BL, H], F32, kind="ExternalOutput")

    MULT = mybir.AluOpType.mult
    ADD = mybir.AluOpType.add
    MAX = mybir.AluOpType.max
    AX = mybir.AxisListType.X
    EXP = mybir.ActivationFunctionType.Exp
    IDN = mybir.ActivationFunctionType.Identity
    CPY = mybir.ActivationFunctionType.Copy

    with tile.TileContext(nc) as tc, ExitStack() as ctx:
        sing = ctx.enter_context(tc.tile_pool(name="sing", bufs=1))
        wpool = ctx.enter_context(tc.tile_pool(name="wpool", bufs=2))
        eop = ctx.enter_context(tc.tile_pool(name="eop", bufs=EO_BUFS))
        prodp = ctx.enter_context(tc.tile_pool(name="prodp", bufs=PROD_BUFS))
        tmpp = ctx.enter_context(tc.tile_pool(name="tmpp", bufs=2))
        invqp = ctx.enter_context(tc.tile_pool(name="invqp", bufs=2))
        qbp = ctx.enter_context(tc.tile_pool(name="qbp", bufs=3))
        alphp = ctx.enter_context(tc.tile_pool(name="alphp", bufs=3))
        smallp = ctx.enter_context(tc.tile_pool(name="smallp", bufs=3))
        outp = ctx.enter_context(tc.tile_pool(name="outp", bufs=1))
        psS = ctx.enter_context(tc.tile_pool(name="psS", bufs=4, space="PSUM"))
        psB = ctx.enter_context(tc.tile_pool(name="psB", bufs=2, space="PSUM"))
        dramp = ctx.enter_context(tc.tile_pool(name="dramp", bufs=1, space="DRAM"))

        # ---- constants
        ident = sing.tile([128, 128], F32)
        make_identity(nc, ident[:])
        one1 = sing.tile([128, 1], F32)
        nc.vector.memset(one1[:], 1.0)

        # ---- W^T: wt[:, h, o] = W[o, h]  (h on partitions)
        # 4 TensorE transposes packed per PSUM bank -> 1 scalar copy each
        wt = sing.tile([128, HC, H], F32)
        for o in range(HC):
            wrow = wpool.tile([128, H], F32, tag="wrow")
            nc.sync.dma_start(wrow[:], w_d[bass.ts(o, 128), :])
            for hb in range(2):
                p4 = psS.tile([128, 4, 128], F32, tag="psS")
                for j in range(4):
                    h = hb * 4 + j
                    nc.tensor.transpose(p4[:, j, :], wrow[:, bass.ts(h, 128)], ident[:])
                nc.vector.tensor_copy(wt[:, hb * 4 : hb * 4 + 4, bass.ts(o, 128)], p4[:])

        # ---- hidden^T, per h-chunk [128h, BL]
        hid = sing.tile([BL, H], F32)
        nc.sync.dma_start(hid[:], hid_d[:])
        pH = psS.tile([128, HC, BL], F32, tag="psS")
        for h in range(HC):
            nc.tensor.transpose(pH[:, h, :], hid[:, bass.ts(h, 128)], ident[0:BL, 0:BL])
        hT = sing.tile([128, HC, BL], F32)
        nc.scalar.copy(hT[:], pH[:])

        # ---- q = hidden @ W.T   (f32 for alphas precision)
        q_ps = psB.tile([BL, H], F32, tag="big")
        for nh in range(H // 512):
            for h in range(HC):
                nc.tensor.matmul(
                    q_ps[:, bass.ts(nh, 512)],
                    hT[:, h, :],
                    wt[:, h, bass.ts(nh, 512)],
                    start=(h == 0),
                    stop=(h == HC - 1),
                )
        q_sb = sing.tile([BL, H], F32)
        nc.scalar.copy(q_sb[:], q_ps[:])
        # bounce q through DRAM so it can be partition-broadcast per batch
        q_dram = dramp.tile([BL, H], F32)
        nc.sync.dma_start(q_dram[:], q_sb[:])
        invq_all = sing.tile([BL, H], F32)
        nc.vector.reciprocal(invq_all[:], q_sb[:])
        invq_dram = dramp.tile([BL, H], F32)
        nc.sync.dma_start(invq_dram[:], invq_all[:])

        # ---- mask, all batches at once: [SC, BL, 128] (partition = chunk)
        mk_all = sing.tile([SC, BL, 128], F32)
        nc.gpsimd.dma_start(mk_all[:], mk_d.rearrange("b (c p) -> c b p", p=128))
        onem_all = sing.tile([SC, BL, 128], F32)
        nc.scalar.activation(
            out=onem_all[:], in_=mk_all[:],
            func=IDN, bias=one1[0:SC, :], scale=-1.0,
        )

        # ---- main loop over local batches
        # qb broadcasts are issued one batch ahead on GpSimd so they are
        # never queued behind the out-DMA's wait in the SWDGE FIFO.
        qb_tiles = {}

        def issue_qb(bb):
            t = qbp.tile([128, H], F32, tag="qb")
            nc.gpsimd.dma_start(t[:], q_dram[bb : bb + 1, :].to_broadcast([128, H]))
            qb_tiles[bb] = t

        issue_qb(0)
        for b in range(BL):
            if b + 1 < BL:
                issue_qb(b + 1)
            qb = qb_tiles.pop(b)

            alphas = alphp.tile([128, SC], F32, tag="alphas")
            prods = []
            for c in range(SC):
                x = eop.tile([128, H], F32, tag="x")
                nc.sync.dma_start(x[:], eo_d[b, bass.ts(c, 128), :])
                ptmp = tmpp.tile([128, H], F32, tag="ptmp")
                nc.vector.tensor_mul(ptmp[:], x[:], qb[:])
                prod = prodp.tile([128, H], BF16, tag="prod")
                # ScalarE: cast product to bf16 AND reduce alphas in one pass
                nc.scalar.activation(
                    out=prod[:], in_=ptmp[:], func=CPY,
                    bias=0.0, scale=1.0,
                    accum_out=alphas[:, c : c + 1],
                )
                prods.append(prod)

            # two-level softmax: raw per-row max m1 keeps exps <= 1; the
            # transposed side computes the global fix-up g = exp(m1 - mx).
            m1 = smallp.tile([128, 1], F32, tag="m1")
            nc.vector.tensor_reduce(out=m1[:], in_=alphas[:], axis=AX, op=MAX)
            negm1 = smallp.tile([128, 1], F32, tag="negm1")
            nc.scalar.mul(negm1[:], m1[:], -1.0)
            e = smallp.tile([128, SC], F32, tag="e")
            nc.scalar.activation(out=e[:], in_=alphas[:], func=EXP,
                                 bias=negm1[:], scale=1.0)
            onemp = psS.tile([128, SC], F32, tag="psS")
            nc.tensor.transpose(onemp[:], onem_all[:, b, :], ident[0:SC, 0:SC])
            u = smallp.tile([128, SC], F32, tag="u")
            nc.vector.tensor_mul(u[:], e[:], onemp[:])
            s1 = smallp.tile([128, 1], F32, tag="s1")
            nc.vector.tensor_reduce(out=s1[:], in_=u[:], axis=AX, op=ADD)

            m1p = psS.tile([1, 128], F32, tag="psS")
            nc.tensor.transpose(m1p[:], m1[:], ident[:])
            s1p = psS.tile([1, 128], F32, tag="psS")
            nc.tensor.transpose(s1p[:], s1[:], ident[:])
            mx = smallp.tile([1, 1], F32, tag="mx")
            nc.vector.tensor_reduce(out=mx[:], in_=m1p[:], axis=AX, op=MAX)
            negmx = smallp.tile([1, 1], F32, tag="negmx")
            nc.scalar.mul(negmx[:], mx[:], -1.0)
            g = smallp.tile([1, 128], F32, tag="g")
            nc.scalar.activation(out=g[:], in_=m1p[:], func=EXP,
                                 bias=negmx[0:1, :], scale=1.0)
            w = smallp.tile([1, 128], F32, tag="w")
            nc.vector.tensor_mul(w[:], s1p[:], g[:])
            den = smallp.tile([1, 1], F32, tag="den")
            nc.vector.tensor_reduce(out=den[:], in_=w[:], axis=AX, op=ADD)
            r = smallp.tile([1, 1], F32, tag="r")
            nc.vector.reciprocal(r[:], den[:])
            gps = psS.tile([128, 1], F32, tag="psS")
            nc.tensor.transpose(gps[:], g[:], ident[0:1, 0:1])
            gp = smallp.tile([128, 1], F32, tag="gp")
            nc.scalar.copy(gp[:], gps[:])
            us = smallp.tile([128, SC], BF16, tag="us")
            nc.vector.tensor_scalar_mul(us[:], u[:], gp[:])

            # c*q = sum_c us[:, c]^T @ prod_c   (bf16, full PE rate)
            c_ps = psB.tile([1, H], F32, tag="big")
            for c in range(SC):
                for nh in range(H // 512):
                    nc.tensor.matmul(
                        c_ps[0:1, bass.ts(nh, 512)],
                        us[:, c : c + 1],
                        prods[c][:, bass.ts(nh, 512)],
                        start=(c == 0),
                        stop=(c == SC - 1),
                    )
            # c = (c*q) * r / q ; the bf16 error enters linearly only
            invqb = invqp.tile([1, H], F32, tag="invqb")
            nc.gpsimd.dma_start(invqb[:], invq_dram[b : b + 1, :])
            final = outp.tile([1, H], F32, tag="final")
            nc.vector.scalar_tensor_tensor(
                out=final[:], in0=c_ps[:], scalar=r[0:1, 0:1],
                in1=invqb[:], op0=MULT, op1=MULT)
            nc.gpsimd.dma_start(out_d[b : b + 1, :], final[:])

    nc.compile()
    return nc


_CACHE = {}


def _get_nc():
    if "nc" not in _CACHE:
        _CACHE["nc"] = _build()
    return _CACHE["nc"]


def _make_in_maps(hidden, encoder_output, encoder_mask, W):
    hidden = np.ascontiguousarray(hidden, dtype=np.float32)
    eo = np.ascontiguousarray(encoder_output, dtype=np.float32)
    mk = np.ascontiguousarray(
        encoder_mask.reshape(B, S).astype(np.float32)
    )
    W = np.ascontiguousarray(W, dtype=np.float32)
    in_maps = []
    for i in range(N_CORES):
        sl = slice(i * BL, (i + 1) * BL)
        in_maps.append(
            {
                "hidden": hidden[sl],
                "encoder_output": eo[sl],
                "encoder_mask": mk[sl],
                "W": W,
            }
        )
    return in_maps


def run(hidden, encoder_output, encoder_mask, W, trace=False):
    nc = _get_nc()
    in_maps = _make_in_maps(hidden, encoder_output, encoder_mask, W)
    res = run_bass_kernel_spmd(nc, in_maps, list(range(N_CORES)), trace=trace)
    out = np.concatenate([res.results[i]["out"] for i in range(N_CORES)], axis=0)
    return out, res


def kernel(hidden, encoder_output, encoder_mask, W):
    out, _ = run(hidden, encoder_output, encoder_mask, W, trace=False)
    return out


# revision 10
# speedup vs baseline: 1.5376x; 1.0938x over previous
"""Masked-softmax attention pooling on 8 TRN2 NeuronCores.

Reference computation (per batch b):
    q = hidden @ W.T                      # [H]
    alphas[s] = eo[b, s, :] . q           # [S]
    alphas = where(mask, -1e16, alphas)
    scores = softmax(alphas)              # over S
    out[b] = sum_s scores[s] * eo[b, s, :]

Sharding: data-parallel over batch (8 batches/core), W replicated.
encoder_output dominates traffic (64 MiB/core) and is streamed from HBM
exactly once in its natural [S, H] layout:
  - alphas via ONE fused DVE op per 128-row chunk (tensor_tensor_reduce:
    accum_out = sum_h x*q)
  - the fused op also emits prod = x*q in bf16; the weighted sum runs
    on TensorE over prod at full bf16 column rate (4x faster than
    fp32), and the output row is divided by q at the end:
    sum_s w_s*(x_s*q)/q == sum_s w_s*x_s (bf16 error enters linearly)
"""

from contextlib import ExitStack

import numpy as np

import concourse.bass as bass
import concourse.tile as tile
from concourse import bacc, mybir
from concourse._compat import get_trn_type
from concourse.bass_utils import run_bass_kernel_spmd
from concourse.masks import make_identity

B, S, H = 64, 2048, 1024
N_CORES = 8
BL = B // N_CORES      # 8 batches per core
SC = S // 128          # 16 s-chunks per batch
HC = H // 128          # 8 h-chunks
F32 = mybir.dt.float32
F32R = mybir.dt.float32r
BF16 = mybir.dt.bfloat16

FUSED_DVE = True       # one DVE scalar_tensor_tensor per chunk (accum_out)
EO_BUFS = 12 if FUSED_DVE else 10  # x is freed right after the chunk op
PROD_BUFS = 32         # bf16 product tiles; live until the batch's matmuls


def _build():
    nc = bacc.Bacc(get_trn_type() or "TRN2", target_bir_lowering=False)

    hid_d = nc.dram_tensor("hidden", [BL, H], F32, kind="ExternalInput")
    eo_d = nc.dram_tensor("encoder_output", [BL, S, H], F32, kind="ExternalInput")
    mk_d = nc.dram_tensor("encoder_mask", [BL, S], F32, kind="ExternalInput")
    w_d = nc.dram_tensor("W", [H, H], F32, kind="ExternalInput")
    out_d = nc.dram_tensor("out", [BL, H], F32, kind="ExternalOutput")

    MULT = mybir.AluOpType.mult
    ADD = mybir.AluOpType.add
    MAX = mybir.AluOpType.max
    AX = mybir.AxisListType.X
    EXP = mybir.ActivationFunctionType.Exp
    IDN = mybir.ActivationFunctionType.Identity
    CPY = mybir.ActivationFunctionType.Copy

    with tile.TileContext(nc) as tc, ExitStack() as ctx:
        sing = ctx.enter_context(tc.tile_pool(name="sing", bufs=1))
        wpool = ctx.enter_context(tc.tile_pool(name="wpool", bufs=2))
        eop = ctx.enter_context(tc.tile_pool(name="eop", bufs=EO_BUFS))
        prodp = ctx.enter_context(tc.tile_pool(name="prodp", bufs=PROD_BUFS))
        tmpp = ctx.enter_context(tc.tile_pool(name="tmpp", bufs=2))
        invqp = ctx.enter_context(tc.tile_pool(name="invqp", bufs=2))
        qbp = ctx.enter_context(tc.tile_pool(name="qbp", bufs=3))
        alphp = ctx.enter_context(tc.tile_pool(name="alphp", bufs=3))
        smallp = ctx.enter_context(tc.tile_pool(name="smallp", bufs=3))
        outp = ctx.enter_context(tc.tile_pool(name="outp", bufs=1))
        psS = ctx.enter_context(tc.tile_pool(name="psS", bufs=4, space="PSUM"))
        psB = ctx.enter_context(tc.tile_pool(name="psB", bufs=2, space="PSUM"))
        dramp = ctx.enter_context(tc.tile_pool(name="dramp", bufs=1, space="DRAM"))

        # ---- constants
        ident = sing.tile([128, 128], F32)
        make_identity(nc, ident[:])
        one1 = sing.tile([128, 1], F32)
        nc.vector.memset(one1[:], 1.0)

        # ---- hidden^T, per h-chunk [128h, BL]
        hid = sing.tile([BL, H], F32)
        nc.sync.dma_start(hid[:], hid_d[:])
        pH = psS.tile([128, HC, BL], F32, tag="psS")
        for h in range(HC):
            nc.tensor.transpose(pH[:, h, :], hid[:, bass.ts(h, 128)], ident[0:BL, 0:BL])
        hT = sing.tile([128, HC, BL], F32)
        nc.scalar.copy(hT[:], pH[:])

        # ---- W^T (wt[:, h, o] = W[o, h]) and q = hidden @ W.T, pipelined
        # per W row-block: each o-column-group of q accumulates as soon as
        # its wrow is transposed, instead of waiting for all of W.
        wt = sing.tile([128, HC, H], F32)
        q_ps = psB.tile([BL, H], F32, tag="big")
        for o in range(HC):
            wrow = wpool.tile([128, H], F32, tag="wrow")
            nc.sync.dma_start(wrow[:], w_d[bass.ts(o, 128), :])
            for hb in range(2):
                p4 = psS.tile([128, 4, 128], F32, tag="psS")
                for j in range(4):
                    h = hb * 4 + j
                    nc.tensor.transpose(p4[:, j, :], wrow[:, bass.ts(h, 128)], ident[:])
                nc.vector.tensor_copy(wt[:, hb * 4 : hb * 4 + 4, bass.ts(o, 128)], p4[:])
            for h in range(HC):
                nc.tensor.matmul(
                    q_ps[:, bass.ts(o, 128)],
                    hT[:, h, :],
                    wt[:, h, bass.ts(o, 128)],
                    start=(h == 0),
                    stop=(h == HC - 1),
                )
        q_sb = sing.tile([BL, H], F32)
        nc.scalar.copy(q_sb[:], q_ps[:])
        # bounce q through DRAM so it can be partition-broadcast per batch
        q_dram = dramp.tile([BL, H], F32)
        nc.sync.dma_start(q_dram[:], q_sb[:])
        invq_all = sing.tile([BL, H], F32)
        nc.vector.reciprocal(invq_all[:], q_sb[:])
        invq_dram = dramp.tile([BL, H], F32)
        nc.sync.dma_start(invq_dram[:], invq_all[:])

        # ---- mask, all batches at once: [SC, BL, 128] (partition = chunk)
        mk_all = sing.tile([SC, BL, 128], F32)
        nc.gpsimd.dma_start(mk_all[:], mk_d.rearrange("b (c p) -> c b p", p=128))
        onem_all = sing.tile([SC, BL, 128], F32)
        nc.scalar.activation(
            out=onem_all[:], in_=mk_all[:],
            func=IDN, bias=one1[0:SC, :], scale=-1.0,
        )

        # ---- main loop over local batches
        # qb broadcasts are issued one batch ahead on GpSimd so they are
        # never queued behind the out-DMA's wait in the SWDGE FIFO.
        qb_tiles = {}

        def issue_qb(bb):
            t = qbp.tile([128, H], F32, tag="qb")
            nc.gpsimd.dma_start(t[:], q_dram[bb : bb + 1, :].to_broadcast([128, H]))
            qb_tiles[bb] = t

        issue_qb(0)
        for b in range(BL):
            if b + 1 < BL:
                issue_qb(b + 1)
            qb = qb_tiles.pop(b)

            alphas = alphp.tile([128, SC], F32, tag="alphas")
            prods = []
            for c in range(SC):
                x = eop.tile([128, H], F32, tag="x")
                nc.sync.dma_start(x[:], eo_d[b, bass.ts(c, 128), :])
                prod = prodp.tile([128, H], BF16, tag="prod")
                if FUSED_DVE:
                    # one DVE pass: prod = x*qb (bf16), alphas[:, c] = sum
                    nc.vector.scalar_tensor_tensor(
                        out=prod[:], in0=x[:], scalar=1.0, in1=qb[:],
                        op0=MULT, op1=MULT,
                        accum_out=alphas[:, c : c + 1],
                    )
                else:
                    ptmp = tmpp.tile([128, H], F32, tag="ptmp")
                    nc.vector.tensor_mul(ptmp[:], x[:], qb[:])
                    # ScalarE: cast to bf16 AND reduce alphas in one pass
                    nc.scalar.activation(
                        out=prod[:], in_=ptmp[:], func=CPY,
                        bias=0.0, scale=1.0,
                        accum_out=alphas[:, c : c + 1],
                    )
                prods.append(prod)

            # two-level softmax: raw per-row max m1 keeps exps <= 1; the
            # transposed side computes the global fix-up g = exp(m1 - mx).
            m1 = smallp.tile([128, 1], F32, tag="m1")
            nc.vector.tensor_reduce(out=m1[:], in_=alphas[:], axis=AX, op=MAX)
            negm1 = smallp.tile([128, 1], F32, tag="negm1")
            nc.scalar.mul(negm1[:], m1[:], -1.0)
            e = smallp.tile([128, SC], F32, tag="e")
            nc.scalar.activation(out=e[:], in_=alphas[:], func=EXP,
                                 bias=negm1[:], scale=1.0)
            onemp = psS.tile([128, SC], F32, tag="psS")
            nc.tensor.transpose(onemp[:], onem_all[:, b, :], ident[0:SC, 0:SC])
            u = smallp.tile([128, SC], F32, tag="u")
            nc.vector.tensor_mul(u[:], e[:], onemp[:])
            s1 = smallp.tile([128, 1], F32, tag="s1")
            nc.vector.tensor_reduce(out=s1[:], in_=u[:], axis=AX, op=ADD)

            m1p = psS.tile([1, 128], F32, tag="psS")
            nc.tensor.transpose(m1p[:], m1[:], ident[:])
            s1p = psS.tile([1, 128], F32, tag="psS")
            nc.tensor.transpose(s1p[:], s1[:], ident[:])
            mx = smallp.tile([1, 1], F32, tag="mx")
            nc.vector.tensor_reduce(out=mx[:], in_=m1p[:], axis=AX, op=MAX)
            negmx = smallp.tile([1, 1], F32, tag="negmx")
            nc.scalar.mul(negmx[:], mx[:], -1.0)
            g = smallp.tile([1, 128], F32, tag="g")
            nc.scalar.activation(out=g[:], in_=m1p[:], func=EXP,
                                 bias=negmx[0:1, :], scale=1.0)
            w = smallp.tile([1, 128], F32, tag="w")
            nc.vector.tensor_mul(w[:], s1p[:], g[:])
            den = smallp.tile([1, 1], F32, tag="den")
            nc.vector.tensor_reduce(out=den[:], in_=w[:], axis=AX, op=ADD)
            r = smallp.tile([1, 1], F32, tag="r")
            nc.vector.reciprocal(r[:], den[:])
            gps = psS.tile([128, 1], F32, tag="psS")
            nc.tensor.transpose(gps[:], g[:], ident[0:1, 0:1])
            gp = smallp.tile([128, 1], F32, tag="gp")
            nc.scalar.copy(gp[:], gps[:])
            us = smallp.tile([128, SC], BF16, tag="us")
            nc.vector.tensor_scalar_mul(us[:], u[:], gp[:])

            # c*q = sum_c us[:, c]^T @ prod_c   (bf16, full PE rate)
            c_ps = psB.tile([1, H], F32, tag="big")
            for c in range(SC):
                for nh in range(H // 512):
                    nc.tensor.matmul(
                        c_ps[0:1, bass.ts(nh, 512)],
                        us[:, c : c + 1],
                        prods[c][:, bass.ts(nh, 512)],
                        start=(c == 0),
                        stop=(c == SC - 1),
                    )
            # c = (c*q) * r / q ; the bf16 error enters linearly only
            invqb = invqp.tile([1, H], F32, tag="invqb")
            nc.gpsimd.dma_start(invqb[:], invq_dram[b : b + 1, :])
            final = outp.tile([1, H], F32, tag="final")
            nc.vector.scalar_tensor_tensor(
                out=final[:], in0=c_ps[:], scalar=r[0:1, 0:1],
                in1=invqb[:], op0=MULT, op1=MULT)
            nc.gpsimd.dma_start(out_d[b : b + 1, :], final[:])

    nc.compile()
    return nc


_CACHE = {}


def _get_nc():
    if "nc" not in _CACHE:
        _CACHE["nc"] = _build()
    return _CACHE["nc"]


def _make_in_maps(hidden, encoder_output, encoder_mask, W):
    hidden = np.ascontiguousarray(hidden, dtype=np.float32)
    eo = np.ascontiguousarray(encoder_output, dtype=np.float32)
    mk = np.ascontiguousarray(
        encoder_mask.reshape(B, S).astype(np.float32)
    )
    W = np.ascontiguousarray(W, dtype=np.float32)
    in_maps = []
    for i in range(N_CORES):
        sl = slice(i * BL, (i + 1) * BL)
        in_maps.append(
            {
                "hidden": hidden[sl],
                "encoder_output": eo[sl],
                "encoder_mask": mk[sl],
                "W": W,
            }
        )
    return in_maps


def run(hidden, encoder_output, encoder_mask, W, trace=False):
    nc = _get_nc()
    in_maps = _make_in_maps(hidden, encoder_output, encoder_mask, W)
    res = run_bass_kernel_spmd(nc, in_maps, list(range(N_CORES)), trace=trace)
    out = np.concatenate([res.results[i]["out"] for i in range(N_CORES)], axis=0)
    return out, res


def kernel(hidden, encoder_output, encoder_mask, W):
    out, _ = run(hidden, encoder_output, encoder_mask, W, trace=False)
    return out
